# revision 23
# baseline (speedup 1.0000x reference)
"""EquiConv (DeepH-E3) Trainium2 kernel — 8-core data-parallel over edges.

Strategy (channel-major, fp16 on-device, 4-deep software pipeline):
  - Host folds per-channel weights/constants into fp16 matmul matrices,
    shards edges across 8 cores, pads to 49x512 per core, and packs all
    ten per-tile input row-blocks (x1 scalars, x1 vectors, fea_weight,
    and the 128-replicated per-edge x2 scalars) into ONE interleaved
    DRAM tensor so each tile needs a single load DMA (prefetched two
    iterations ahead).
  - The P2 path uses zero-padded weight columns ([w2|0], [0|w2]) so both
    vector components accumulate into one diag(w3,w3)-anchored PSUM
    group; gate and MLP-wb outputs are column-duplicated so the sigmoid
    gate covers all 128 partitions in single ops.
  - Emission is software-pipelined four tiles deep so every PE matmul's
    inputs are ready >=1 iteration ahead: the PE stays dense and ramps
    to its max p-state (~216-330ns per 512-col fp16 matmul vs 427ns).
  - All SBUF traffic is fp16 (2x DVE rate, operands packed pairwise
    into 1024-col fused DVE ops via stride-0 repeat APs); PSUM stays
    f32.  ACT does the PSUM->SBUF crossings; sigmoid uses tanh(g/2).
  - Output is written fp16 channel-major [320, E] and unpacked on host.

Self-contained: hardcodes shapes from the problem spec; no file reads.
"""
import os
import sys

import numpy as np

# ---------------------------------------------------------------- constants
E_FULL = 200000
N_CORES = 8
E_CORE = E_FULL // N_CORES      # 25000
NT = 512                        # edges per tile
T_TILES = 49                    # tiles per core
E_PAD = NT * T_TILES            # 25088
MUL_S = 128
MUL_V = 64
N_BLK = 10                      # input row-blocks per tile
TILE_COLS = N_BLK * NT          # 5120

INV_S = 1.0 / np.sqrt(MUL_S)
INV_V = 1.0 / np.sqrt(MUL_V)
SQ2 = 1.0 / np.sqrt(2.0)
SQ3 = 1.0 / np.sqrt(3.0)

_REPO_CANDIDATES = (
    "/opt/trn_rl_repo",
    "/root/.axon_site/_ro/trn_rl_repo",
)


def _ensure_repo_on_path():
    try:
        import concourse.bass  # noqa: F401
        return
    except ImportError:
        pass
    for p in _REPO_CANDIDATES:
        if os.path.isdir(p) and p not in sys.path:
            sys.path.insert(0, p)
    import concourse.bass  # noqa: F401


_CACHE = {}


def _build_nc():
    """Build + compile the per-core Bass program (cached)."""
    if "nc" in _CACHE:
        return _CACHE["nc"]
    _ensure_repo_on_path()
    import concourse.mybir as mybir
    import concourse.tile as tile
    from concourse import bacc

    F32 = mybir.dt.float32
    F16 = mybir.dt.float16
    MULT = mybir.AluOpType.mult
    ADD = mybir.AluOpType.add
    AF = mybir.ActivationFunctionType

    nc = bacc.Bacc(trn_type="TRN2", target_bir_lowering=False, debug=False,
                   num_devices=N_CORES)

    # DRAM inputs (per-core shard) ----------------------------------------
    d_in = nc.dram_tensor("in_t", [128, T_TILES * TILE_COLS], F16,
                          kind="ExternalInput")
    d_wa0 = nc.dram_tensor("wa0", [128, 128], F16, kind="ExternalInput")
    d_wa1d = nc.dram_tensor("wa1d", [128, 128], F16, kind="ExternalInput")
    d_w2lo = nc.dram_tensor("w2lo", [128, 128], F16, kind="ExternalInput")
    d_w2hi = nc.dram_tensor("w2hi", [128, 128], F16, kind="ExternalInput")
    d_w2v2 = nc.dram_tensor("w2v2", [128, 64], F16, kind="ExternalInput")
    d_wb4s = nc.dram_tensor("wb4s", [128, 128], F16, kind="ExternalInput")
    d_wb5sd = nc.dram_tensor("wb5sd", [128, 128], F16, kind="ExternalInput")
    d_wb4b = nc.dram_tensor("wb4b", [64, 128], F16, kind="ExternalInput")
    d_wb5bd = nc.dram_tensor("wb5bd", [64, 128], F16, kind="ExternalInput")
    d_wcdiag = nc.dram_tensor("wcdiag", [128, 128], F16, kind="ExternalInput")
    d_wc2 = nc.dram_tensor("wc2", [64, 64], F16, kind="ExternalInput")
    d_fc0 = nc.dram_tensor("fc0", [128, 64], F16, kind="ExternalInput")
    d_fc1 = nc.dram_tensor("fc1", [64, 64], F16, kind="ExternalInput")
    d_fc2a = nc.dram_tensor("fc2a", [64, 128], F16, kind="ExternalInput")
    d_fc2bd = nc.dram_tensor("fc2bd", [64, 128], F16, kind="ExternalInput")
    d_b0 = nc.dram_tensor("b0c", [64, 1], F32, kind="ExternalInput")
    d_b1 = nc.dram_tensor("b1c", [64, 1], F32, kind="ExternalInput")
    d_b2a = nc.dram_tensor("b2a", [128, 1], F32, kind="ExternalInput")
    d_b2bh = nc.dram_tensor("b2bh", [128, 1], F32, kind="ExternalInput")

    d_out = nc.dram_tensor("out_t", [320, E_PAD], F16, kind="ExternalOutput")

    with tile.TileContext(nc) as tc:
        with tc.tile_pool(name="const", bufs=1) as cp, \
             tc.tile_pool(name="io", bufs=4) as io, \
             tc.tile_pool(name="work", bufs=2) as wk, \
             tc.tile_pool(name="ps", bufs=1, space="PSUM") as ps:

            def const(d, shape, dtype=F16):
                t = cp.tile(shape, dtype, name=d.name + "_sb")
                nc.sync.dma_start(t, d.ap())
                return t

            w_wa0 = const(d_wa0, [128, 128])
            w_wa1d = const(d_wa1d, [128, 128])
            w_w2lo = const(d_w2lo, [128, 128])
            w_w2hi = const(d_w2hi, [128, 128])
            w_w2v2 = const(d_w2v2, [128, 64])
            w_wb4s = const(d_wb4s, [128, 128])
            w_wb5sd = const(d_wb5sd, [128, 128])
            w_wb4b = const(d_wb4b, [64, 128])
            w_wb5bd = const(d_wb5bd, [64, 128])
            w_wcdiag = const(d_wcdiag, [128, 128])
            w_fc0 = const(d_fc0, [128, 64])
            w_fc1 = const(d_fc1, [64, 64])
            w_fc2a = const(d_fc2a, [64, 128])
            w_fc2bd = const(d_fc2bd, [64, 128])
            c_b0 = const(d_b0, [64, 1], F32)
            c_b1 = const(d_b1, [64, 1], F32)
            c_b2a = const(d_b2a, [128, 1], F32)
            c_b2bh = const(d_b2bh, [128, 1], F32)
            # w3f copy at partitions 64-127 for the row-offset matmul
            w_wc_f = cp.tile([128, 64], F16, name="wc_hi_sb")
            w_wc_hi = w_wc_f[64:128, :]
            nc.sync.dma_start(w_wc_hi, d_wc2.ap())

            # per-tile state rings, keyed by tile index
            S = {}

            def load(k):
                csl = slice(k * TILE_COLS, (k + 1) * TILE_COLS)
                mega = io.tile([128, TILE_COLS], F16)
                nc.sync.dma_start(mega, d_in.ap()[:, csl])
                S[k] = {"mega": mega}

            def mega_slices(k):
                m = S[k]["mega"]
                return {
                    "x1s": m[:, 0 * NT:1 * NT],
                    "x1va": m[:, 1 * NT:2 * NT],
                    "x2d": m[:, 2 * NT:3 * NT],
                    "fwt": m[:, 3 * NT:4 * NT],
                    "rp01": m[:, 4 * NT:6 * NT],   # [r_v01 | r_s]
                    "rsv0": m[:, 5 * NT:7 * NT],   # [r_s | r_v0]
                    "rv12": m[:, 7 * NT:9 * NT],   # [r_v1 | r_v2]
                    "r_sv2": m[:, 9 * NT:10 * NT],
                }

            def rep2(ap):
                """Stride-0 repeat of a [128, NT] slice -> [128, 2, NT]."""
                return ap.unsqueeze(1).broadcast_to([128, 2, NT])

            def prep_dve(k):
                sk = S[k]
                ms = mega_slices(k)
                f1 = wk.tile([128, 2 * NT], F16)
                nc.vector.tensor_tensor(
                    f1.rearrange("p (b c) -> p b c", b=2),
                    rep2(ms["x1va"]), ms["rp01"].rearrange(
                        "p (b c) -> p b c", b=2), MULT)
                sk.update(xv_p01=f1[:, 0:NT], xv_s01=f1[:, NT:2 * NT])
                f2 = wk.tile([128, 2 * NT], F16)
                nc.vector.tensor_tensor(
                    f2.rearrange("p (b c) -> p b c", b=2),
                    rep2(ms["x1s"]), ms["rsv0"].rearrange(
                        "p (b c) -> p b c", b=2), MULT)
                f3 = wk.tile([128, 2 * NT], F16)
                nc.vector.tensor_tensor(
                    f3.rearrange("p (b c) -> p b c", b=2),
                    rep2(ms["x1s"]), ms["rv12"].rearrange(
                        "p (b c) -> p b c", b=2), MULT)
                xv_sp2 = wk.tile([128, NT], F16)
                nc.gpsimd.tensor_tensor(xv_sp2, ms["x2d"], ms["r_sv2"], MULT)
                sk.update(x1s_s=f2[:, 0:NT], x1s_v0=f2[:, NT:2 * NT],
                          x1s_v1=f3[:, 0:NT], x1s_v2=f3[:, NT:2 * NT],
                          xv_sp2=xv_sp2)

            def prep(k):
                """MLP front (h1) for tile k."""
                sk = S[k]
                ms = mega_slices(k)
                h1 = ps.tile([64, NT], F32, tag="h1")
                nc.tensor.matmul(h1, w_fc0, ms["fwt"], start=True, stop=True)
                h1s = wk.tile([64, NT], F16)
                nc.scalar.activation(h1s, h1, AF.Silu, bias=c_b0)
                sk.update(h1s=h1s)

            def prep_h2(k):
                sk = S[k]
                h2 = ps.tile([64, NT], F32, tag="h2")
                nc.tensor.matmul(h2, w_fc1, sk["h1s"], start=True, stop=True)
                h2s = wk.tile([64, NT], F16)
                nc.scalar.activation(h2s, h2, AF.Silu, bias=c_b1)
                sk.update(h2s=h2s)

            def main_mlp2(k):
                """wwa / wwb matmuls + crossings for tile k."""
                sk = S[k]
                wwa = ps.tile([128, NT], F32, tag="wwa")
                nc.tensor.matmul(wwa, w_fc2a, sk["h2s"], start=True, stop=True)
                wwb = ps.tile([128, NT], F32, tag="wwb")
                nc.tensor.matmul(wwb, w_fc2bd, sk["h2s"],
                                 start=True, stop=True)
                wbs = wk.tile([128, NT], F16)
                nc.scalar.activation(wbs, wwb, AF.Identity, bias=c_b2bh)
                was = wk.tile([128, NT], F16)
                nc.scalar.activation(was, wwa, AF.Identity, bias=c_b2a)
                sk.update(wbs=wbs, was=was)

            def main_tp_a(k):
                """Six accumulating TP matmuls for tile k."""
                sk = S[k]
                scal = ps.tile([128, NT], F32, tag="scal")
                gate2 = ps.tile([128, NT], F32, tag="gate2")
                nc.tensor.matmul(scal, w_wa0, sk["x1s_s"],
                                 start=True, stop=False)
                nc.tensor.matmul(gate2, w_wa1d, sk["x1s_s"],
                                 start=True, stop=False)
                nc.tensor.matmul(scal, w_wb4s, sk["xv_p01"],
                                 start=False, stop=False)
                nc.tensor.matmul(gate2, w_wb5sd, sk["xv_p01"],
                                 start=False, stop=False)
                nc.tensor.matmul(scal, w_wb4b, sk["xv_sp2"][0:64, :],
                                 start=False, stop=True)
                nc.tensor.matmul(gate2, w_wb5bd, sk["xv_sp2"][0:64, :],
                                 start=False, stop=True)
                sk.update(scal=scal, gate2=gate2)

            def main_vec(k):
                """vec01 / vec2 accumulation groups."""
                sk = S[k]
                vec01 = ps.tile([128, NT], F32, tag="vec01")
                vec2 = ps.tile([64, NT], F32, tag="vec2")
                nc.tensor.matmul(vec01, w_wcdiag, sk["xv_s01"],
                                 start=True, stop=False)
                nc.tensor.matmul(vec01, w_w2lo, sk["x1s_v0"],
                                 start=False, stop=False)
                nc.tensor.matmul(vec01, w_w2hi, sk["x1s_v1"],
                                 start=False, stop=True)
                nc.tensor.matmul(vec2, w_w2v2, sk["x1s_v2"],
                                 start=True, stop=False)
                nc.tensor.matmul(vec2, w_wc_hi, sk["xv_sp2"][64:128, :],
                                 start=False, stop=True,
                                 tile_position=(64, 0))
                sk.update(vec01=vec01, vec2=vec2)

            def main_act(k):
                sk = S[k]
                sc_silu = wk.tile([128, NT], F16)
                nc.scalar.activation(sc_silu, sk["scal"], AF.Silu)
                tg = wk.tile([128, NT], F16)
                nc.scalar.activation(tg, sk["gate2"], AF.Tanh, scale=0.5)
                sk.update(sc_silu=sc_silu, tg=tg)

            def out_phase(k):
                """Sigmoid chain + output muls + stores for tile k."""
                sk = S[k]
                sl = slice(k * NT, (k + 1) * NT)
                # sgw2 = (tg+1)*wbs = sigmoid(g)*(w+b)
                sgw2 = wk.tile([128, NT], F16)
                nc.vector.scalar_tensor_tensor(sgw2, sk["tg"], 1.0,
                                               sk["wbs"], ADD, MULT)
                out01 = wk.tile([128, NT], F16)
                nc.vector.tensor_tensor(out01, sk["vec01"], sgw2, MULT)
                out2 = wk.tile([64, NT], F16)
                nc.vector.tensor_tensor(out2, sk["vec2"], sgw2[0:64, :], MULT)
                out_s = wk.tile([128, NT], F16)
                nc.vector.tensor_tensor(out_s, sk["sc_silu"], sk["was"], MULT)
                nc.sync.dma_start(d_out.ap()[128:256, sl], out01)
                nc.sync.dma_start(d_out.ap()[256:320, sl], out2)
                nc.sync.dma_start(d_out.ap()[0:128, sl], out_s)

            # ---- pipelined emission (depth 4) ----------------------
            # iteration k: load(k+1) | prep(k) | h2/TP(k-1) | mlp2+out(k-2)
            T = T_TILES
            for k in range(T + 3):
                if k == 0:
                    load(0)
                    load(1)
                if k + 2 < T:
                    load(k + 2)
                m = k - 1        # TP tile
                o = k - 2        # mlp2 + output tile
                if 0 <= o < T:
                    main_mlp2(o)      # PE 1-2; ACT wbs, was
                    out_phase(o)      # DVE sgw2, out01, out2, out_s
                if k < T:
                    prep_dve(k)       # DVE fused prescales
                    prep(k)           # PE 3: h1; ACT h1s
                if 0 <= m < T:
                    prep_h2(m)        # PE 4: h2; ACT h2s
                    main_tp_a(m)      # PE 5-10
                    main_vec(m)       # PE 11-15
                    main_act(m)       # ACT sc_silu, tg
                if o - 1 in S:
                    del S[o - 1]

    nc.compile()
    _CACHE["nc"] = nc
    return nc


def _fold_weights(inp):
    """Fold per-channel weights + constants into fp16 matmul matrices."""
    f = lambda k: np.asarray(inp[k], dtype=np.float32)
    w0f = f("w1_p0") * f("w2_p0")[None, :] * (INV_S * SQ2)
    w1f = f("w1_p1") * f("w2_p1")[None, :] * (INV_S * SQ2)
    w2f = f("w1_p2") * f("w2_p2")[None, :] * (INV_S * SQ2)
    w3f = f("w1_p3") * f("w2_p3")[None, :] * (INV_V * SQ2)
    w4f = f("w1_p4") * f("w2_p4")[None, :] * (INV_V * SQ3 * SQ2)
    w5f = f("w1_p5") * f("w2_p5")[None, :] * (INV_V * SQ3 * SQ2)
    fc2 = f("fc_w2")
    b2 = f("fc_b2")
    w5d = np.concatenate([w5f, w5f], axis=1)         # [64,128] col-dup
    cdiag = np.zeros((128, 128), np.float32)
    cdiag[0:64, 0:64] = w3f
    cdiag[64:128, 64:128] = w3f
    z64 = np.zeros((128, 64), np.float32)
    h = lambda a: np.ascontiguousarray(a.astype(np.float16))
    c = lambda a: np.ascontiguousarray(a.astype(np.float32))
    return {
        "wa0": h(w0f),
        "wa1d": h(np.concatenate([w1f, w1f], axis=1)),
        "w2lo": h(np.concatenate([w2f, z64], axis=1)),
        "w2hi": h(np.concatenate([z64, w2f], axis=1)),
        "w2v2": h(w2f),
        "wb4s": h(np.concatenate([w4f, w4f], axis=0)),
        "wb5sd": h(np.concatenate([w5d, w5d], axis=0)),
        "wb4b": h(w4f),
        "wb5bd": h(w5d),
        "wcdiag": h(cdiag),
        "wc2": h(w3f),
        "fc0": h(f("fc_w0")),
        "fc1": h(f("fc_w1")),
        "fc2a": h(fc2[:, :128]),
        "fc2bd": h(0.5 * np.concatenate([fc2[:, 128:], fc2[:, 128:]],
                                        axis=1)),
        "b0c": c(f("fc_b0")[:, None]),
        "b1c": c(f("fc_b1")[:, None]),
        "b2a": c(b2[:128, None]),
        "b2bh": c(0.5 * np.concatenate([b2[128:], b2[128:]])[:, None]),
    }


def _shard_inputs(inp):
    """Per-core merged fp16 input tensor + tiny x2 scalar rows."""
    fea_in1 = np.asarray(inp["fea_in1"], dtype=np.float32)
    fea_in2 = np.asarray(inp["fea_in2"], dtype=np.float32)
    fea_w = np.asarray(inp["fea_weight"], dtype=np.float32)
    shards = []
    for cidx in range(N_CORES):
        s = slice(cidx * E_CORE, (cidx + 1) * E_CORE)
        x1 = fea_in1[s]
        x2 = fea_in2[s]
        fw = fea_w[s]
        blocks = np.zeros((N_BLK, 128, E_PAD), np.float16)
        blocks[0][:, :E_CORE] = x1[:, :128].T.astype(np.float16)
        xv = x1[:, 128:].reshape(E_CORE, 64, 3).transpose(2, 1, 0)  # [3,64,E]
        blocks[1][0:64, :E_CORE] = xv[0].astype(np.float16)
        blocks[1][64:128, :E_CORE] = xv[1].astype(np.float16)
        blocks[2][0:64, :E_CORE] = xv[2].astype(np.float16)
        blocks[2][64:128, :E_CORE] = xv[2].astype(np.float16)
        blocks[3][:, :E_CORE] = fw.T.astype(np.float16)
        # x2 rows: [s, v0, v1, v2]
        x2p = np.zeros((4, E_PAD), np.float16)
        x2p[0, :E_CORE] = x2[:, 0].astype(np.float16)
        x2p[1, :E_CORE] = x2[:, 1].astype(np.float16)
        x2p[2, :E_CORE] = x2[:, 2].astype(np.float16)
        x2p[3, :E_CORE] = x2[:, 3].astype(np.float16)
        blocks[4][0:64, :] = x2p[1][None, :]      # r_v01 = [v0; v1]
        blocks[4][64:128, :] = x2p[2][None, :]
        blocks[5][:, :] = x2p[0][None, :]         # r_s full
        blocks[6][:, :] = x2p[1][None, :]         # r_v0 full
        blocks[7][:, :] = x2p[2][None, :]         # r_v1 full
        blocks[8][:, :] = x2p[3][None, :]         # r_v2 full
        blocks[9][0:64, :] = x2p[3][None, :]      # r_sv2 = [v2; s]
        blocks[9][64:128, :] = x2p[0][None, :]
        mega = (blocks.reshape(N_BLK, 128, T_TILES, NT)
                .transpose(1, 2, 0, 3)
                .reshape(128, T_TILES * TILE_COLS))
        shards.append({"in_t": np.ascontiguousarray(mega)})
    return shards


def run(inputs, trace=False, trace_kwargs=None):
    """Run the kernel; returns (output [E,320] f32, BassKernelResults)."""
    _ensure_repo_on_path()
    from concourse import bass_utils

    nc = _build_nc()
    weights = _fold_weights(inputs)
    shards = _shard_inputs(inputs)
    in_maps = [{**weights, **sh} for sh in shards]

    kwargs = {}
    if trace:
        _install_ntff_hook()
        kwargs.update(trace=True, **(trace_kwargs or {}))
    res = bass_utils.run_bass_kernel_spmd(
        nc, in_maps, core_ids=list(range(N_CORES)), **kwargs)

    out = np.empty((E_FULL, 320), np.float32)
    for c in range(N_CORES):
        o = res.results[c]["out_t"][:, :E_CORE].astype(np.float32)
        s = slice(c * E_CORE, (c + 1) * E_CORE)
        out[s, :128] = o[:128].T
        out[s, 128:] = (o[128:].reshape(3, 64, E_CORE)
                        .transpose(2, 1, 0).reshape(E_CORE, 192))
    return out, res


def _install_ntff_hook():
    """Shim the missing antenv.axon_hooks so trace=True works under axon."""
    import types
    import antenv
    from concourse import bass_utils
    if "antenv.axon_hooks" in sys.modules:
        return
    mod = types.ModuleType("antenv.axon_hooks")
    _h = [None]
    mod.set_axon_ntff_profile_hook = lambda h: _h.__setitem__(0, h)
    mod.get_axon_ntff_profile_hook = lambda: _h[0]
    sys.modules["antenv.axon_hooks"] = mod
    antenv.axon_hooks = mod
    from trn_agent_boot.trn_boot import _ntff_profile_via_ctypes
    mod.set_axon_ntff_profile_hook(
        _ntff_profile_via_ctypes("/opt/axon/libaxon_pjrt.so"))
    bass_utils.upload_artifacts = lambda tmpdir: tmpdir


def kernel(**inputs) -> np.ndarray:
    out, _ = run(inputs, trace=False)
    return out


# revision 24
# speedup vs baseline: 1.5769x; 1.5769x over previous
"""EquiConv (DeepH-E3) Trainium2 kernel — 8-core data-parallel over edges.

Strategy (channel-major, fp16 on-device, 4-deep software pipeline):
  - Host folds per-channel weights/constants into fp16 matmul matrices,
    shards edges across 8 cores, pads to 49x512 per core, and packs all
    ten per-tile input row-blocks (x1 scalars, x1 vectors, fea_weight,
    and the 128-replicated per-edge x2 scalars) into ONE interleaved
    DRAM tensor so each tile needs a single load DMA (prefetched two
    iterations ahead).
  - The P2 path uses zero-padded weight columns ([w2|0], [0|w2]) so both
    vector components accumulate into one diag(w3,w3)-anchored PSUM
    group; gate and MLP-wb outputs are column-duplicated so the sigmoid
    gate covers all 128 partitions in single ops.
  - Emission is software-pipelined four tiles deep so every PE matmul's
    inputs are ready >=1 iteration ahead: the PE stays dense and ramps
    to its max p-state (~216-330ns per 512-col fp16 matmul vs 427ns).
  - All SBUF traffic is fp16 (2x DVE rate, operands packed pairwise
    into 1024-col fused DVE ops via stride-0 repeat APs); PSUM stays
    f32.  ACT does the PSUM->SBUF crossings; sigmoid uses tanh(g/2).
  - Output is written fp16 channel-major [320, E] and unpacked on host.

Self-contained: hardcodes shapes from the problem spec; no file reads.
"""
import os
import sys

import numpy as np

# ---------------------------------------------------------------- constants
E_FULL = 200000
N_CORES = 8
E_CORE = E_FULL // N_CORES      # 25000
NT = 512                        # edges per tile
T_TILES = 49                    # tiles per core
E_PAD = NT * T_TILES            # 25088
MUL_S = 128
MUL_V = 64
N_BLK = 10                      # input row-blocks per tile
TILE_COLS = N_BLK * NT          # 5120

INV_S = 1.0 / np.sqrt(MUL_S)
INV_V = 1.0 / np.sqrt(MUL_V)
SQ2 = 1.0 / np.sqrt(2.0)
SQ3 = 1.0 / np.sqrt(3.0)

_REPO_CANDIDATES = (
    "/opt/trn_rl_repo",
    "/root/.axon_site/_ro/trn_rl_repo",
)


def _ensure_repo_on_path():
    try:
        import concourse.bass  # noqa: F401
        return
    except ImportError:
        pass
    for p in _REPO_CANDIDATES:
        if os.path.isdir(p) and p not in sys.path:
            sys.path.insert(0, p)
    import concourse.bass  # noqa: F401


_CACHE = {}


def _build_nc():
    """Build + compile the per-core Bass program (cached)."""
    if "nc" in _CACHE:
        return _CACHE["nc"]
    _ensure_repo_on_path()
    import concourse.mybir as mybir
    import concourse.tile as tile
    from concourse import bacc

    F32 = mybir.dt.float32
    F16 = mybir.dt.float16
    MULT = mybir.AluOpType.mult
    ADD = mybir.AluOpType.add
    AF = mybir.ActivationFunctionType

    nc = bacc.Bacc(trn_type="TRN2", target_bir_lowering=False, debug=False,
                   num_devices=N_CORES)

    # DRAM inputs (per-core shard) ----------------------------------------
    d_in = nc.dram_tensor("in_t", [128, T_TILES * TILE_COLS], F16,
                          kind="ExternalInput")
    d_wa0 = nc.dram_tensor("wa0", [128, 128], F16, kind="ExternalInput")
    d_wa1d = nc.dram_tensor("wa1d", [128, 128], F16, kind="ExternalInput")
    d_w2lo = nc.dram_tensor("w2lo", [128, 128], F16, kind="ExternalInput")
    d_w2hi = nc.dram_tensor("w2hi", [128, 128], F16, kind="ExternalInput")
    d_w2v2 = nc.dram_tensor("w2v2", [128, 64], F16, kind="ExternalInput")
    d_wb4s = nc.dram_tensor("wb4s", [128, 128], F16, kind="ExternalInput")
    d_wb5sd = nc.dram_tensor("wb5sd", [128, 128], F16, kind="ExternalInput")
    d_wb4b = nc.dram_tensor("wb4b", [64, 128], F16, kind="ExternalInput")
    d_wb5bd = nc.dram_tensor("wb5bd", [64, 128], F16, kind="ExternalInput")
    d_wcdiag = nc.dram_tensor("wcdiag", [128, 128], F16, kind="ExternalInput")
    d_wc2 = nc.dram_tensor("wc2", [64, 64], F16, kind="ExternalInput")
    d_fc0 = nc.dram_tensor("fc0", [128, 64], F16, kind="ExternalInput")
    d_fc1 = nc.dram_tensor("fc1", [64, 64], F16, kind="ExternalInput")
    d_fc2a = nc.dram_tensor("fc2a", [64, 128], F16, kind="ExternalInput")
    d_fc2bd = nc.dram_tensor("fc2bd", [64, 128], F16, kind="ExternalInput")
    d_b0 = nc.dram_tensor("b0c", [64, 1], F32, kind="ExternalInput")
    d_b1 = nc.dram_tensor("b1c", [64, 1], F32, kind="ExternalInput")
    d_b2a = nc.dram_tensor("b2a", [128, 1], F32, kind="ExternalInput")
    d_b2bh = nc.dram_tensor("b2bh", [128, 1], F32, kind="ExternalInput")

    d_out = nc.dram_tensor("out_t", [320, E_PAD], F16, kind="ExternalOutput")

    with tile.TileContext(nc) as tc:
        with tc.tile_pool(name="const", bufs=1) as cp, \
             tc.tile_pool(name="io", bufs=4) as io, \
             tc.tile_pool(name="work", bufs=2) as wk, \
             tc.tile_pool(name="ps", bufs=1, space="PSUM") as ps:

            def const(d, shape, dtype=F16):
                t = cp.tile(shape, dtype, name=d.name + "_sb")
                nc.sync.dma_start(t, d.ap())
                return t

            w_wa0 = const(d_wa0, [128, 128])
            w_wa1d = const(d_wa1d, [128, 128])
            w_w2lo = const(d_w2lo, [128, 128])
            w_w2hi = const(d_w2hi, [128, 128])
            w_w2v2 = const(d_w2v2, [128, 64])
            w_wb4s = const(d_wb4s, [128, 128])
            w_wb5sd = const(d_wb5sd, [128, 128])
            w_wb4b = const(d_wb4b, [64, 128])
            w_wb5bd = const(d_wb5bd, [64, 128])
            w_wcdiag = const(d_wcdiag, [128, 128])
            w_fc0 = const(d_fc0, [128, 64])
            w_fc1 = const(d_fc1, [64, 64])
            w_fc2a = const(d_fc2a, [64, 128])
            w_fc2bd = const(d_fc2bd, [64, 128])
            c_b0 = const(d_b0, [64, 1], F32)
            c_b1 = const(d_b1, [64, 1], F32)
            c_b2a = const(d_b2a, [128, 1], F32)
            c_b2bh = const(d_b2bh, [128, 1], F32)
            # w3f copy at partitions 64-127 for the row-offset matmul
            w_wc_f = cp.tile([128, 64], F16, name="wc_hi_sb")
            w_wc_hi = w_wc_f[64:128, :]
            nc.sync.dma_start(w_wc_hi, d_wc2.ap())

            # per-tile state rings, keyed by tile index
            S = {}

            def load(k):
                csl = slice(k * TILE_COLS, (k + 1) * TILE_COLS)
                mega = io.tile([128, TILE_COLS], F16)
                nc.sync.dma_start(mega, d_in.ap()[:, csl])
                S[k] = {"mega": mega}

            def mega_slices(k):
                m = S[k]["mega"]
                return {
                    "x1s": m[:, 0 * NT:1 * NT],
                    "x1va": m[:, 1 * NT:2 * NT],
                    "x2d": m[:, 2 * NT:3 * NT],
                    "fwt": m[:, 3 * NT:4 * NT],
                    "rp01": m[:, 4 * NT:6 * NT],   # [r_v01 | r_s]
                    "rsv0": m[:, 5 * NT:7 * NT],   # [r_s | r_v0]
                    "rv12": m[:, 7 * NT:9 * NT],   # [r_v1 | r_v2]
                    "r_sv2": m[:, 9 * NT:10 * NT],
                }

            def rep2(ap):
                """Stride-0 repeat of a [128, NT] slice -> [128, 2, NT]."""
                return ap.unsqueeze(1).broadcast_to([128, 2, NT])

            def prep_dve(k):
                sk = S[k]
                ms = mega_slices(k)
                f1 = wk.tile([128, 2 * NT], F16)
                nc.vector.tensor_tensor(
                    f1.rearrange("p (b c) -> p b c", b=2),
                    rep2(ms["x1va"]), ms["rp01"].rearrange(
                        "p (b c) -> p b c", b=2), MULT)
                sk.update(xv_p01=f1[:, 0:NT], xv_s01=f1[:, NT:2 * NT])
                f2 = wk.tile([128, 2 * NT], F16)
                nc.vector.tensor_tensor(
                    f2.rearrange("p (b c) -> p b c", b=2),
                    rep2(ms["x1s"]), ms["rsv0"].rearrange(
                        "p (b c) -> p b c", b=2), MULT)
                f3 = wk.tile([128, 2 * NT], F16)
                nc.vector.tensor_tensor(
                    f3.rearrange("p (b c) -> p b c", b=2),
                    rep2(ms["x1s"]), ms["rv12"].rearrange(
                        "p (b c) -> p b c", b=2), MULT)
                xv_sp2 = wk.tile([128, NT], F16)
                nc.vector.tensor_tensor(xv_sp2, ms["x2d"], ms["r_sv2"], MULT)
                sk.update(x1s_s=f2[:, 0:NT], x1s_v0=f2[:, NT:2 * NT],
                          x1s_v1=f3[:, 0:NT], x1s_v2=f3[:, NT:2 * NT],
                          xv_sp2=xv_sp2)

            def prep(k):
                """MLP front (h1) for tile k."""
                sk = S[k]
                ms = mega_slices(k)
                h1 = ps.tile([64, NT], F32, tag="h1")
                nc.tensor.matmul(h1, w_fc0, ms["fwt"], start=True, stop=True)
                h1s = wk.tile([64, NT], F16)
                nc.scalar.activation(h1s, h1, AF.Silu, bias=c_b0)
                sk.update(h1s=h1s)

            def prep_h2(k):
                sk = S[k]
                h2 = ps.tile([64, NT], F32, tag="h2")
                nc.tensor.matmul(h2, w_fc1, sk["h1s"], start=True, stop=True)
                h2s = wk.tile([64, NT], F16)
                nc.scalar.activation(h2s, h2, AF.Silu, bias=c_b1)
                sk.update(h2s=h2s)

            def main_mlp2(k):
                """wwa / wwb matmuls + crossings for tile k."""
                sk = S[k]
                wwa = ps.tile([128, NT], F32, tag="wwa")
                nc.tensor.matmul(wwa, w_fc2a, sk["h2s"], start=True, stop=True)
                wwb = ps.tile([128, NT], F32, tag="wwb")
                nc.tensor.matmul(wwb, w_fc2bd, sk["h2s"],
                                 start=True, stop=True)
                wbs = wk.tile([128, NT], F16)
                nc.scalar.activation(wbs, wwb, AF.Identity, bias=c_b2bh)
                was = wk.tile([128, NT], F16)
                nc.scalar.activation(was, wwa, AF.Identity, bias=c_b2a)
                sk.update(wbs=wbs, was=was)

            def main_tp_a(k):
                """Six accumulating TP matmuls for tile k."""
                sk = S[k]
                scal = ps.tile([128, NT], F32, tag="scal")
                gate2 = ps.tile([128, NT], F32, tag="gate2")
                nc.tensor.matmul(scal, w_wa0, sk["x1s_s"],
                                 start=True, stop=False)
                nc.tensor.matmul(gate2, w_wa1d, sk["x1s_s"],
                                 start=True, stop=False)
                nc.tensor.matmul(scal, w_wb4s, sk["xv_p01"],
                                 start=False, stop=False)
                nc.tensor.matmul(gate2, w_wb5sd, sk["xv_p01"],
                                 start=False, stop=False)
                nc.tensor.matmul(scal, w_wb4b, sk["xv_sp2"][0:64, :],
                                 start=False, stop=True)
                nc.tensor.matmul(gate2, w_wb5bd, sk["xv_sp2"][0:64, :],
                                 start=False, stop=True)
                sk.update(scal=scal, gate2=gate2)

            def main_vec(k):
                """vec01 / vec2 accumulation groups."""
                sk = S[k]
                vec01 = ps.tile([128, NT], F32, tag="vec01")
                vec2 = ps.tile([64, NT], F32, tag="vec2")
                nc.tensor.matmul(vec01, w_wcdiag, sk["xv_s01"],
                                 start=True, stop=False)
                nc.tensor.matmul(vec01, w_w2lo, sk["x1s_v0"],
                                 start=False, stop=False)
                nc.tensor.matmul(vec01, w_w2hi, sk["x1s_v1"],
                                 start=False, stop=True)
                nc.tensor.matmul(vec2, w_w2v2, sk["x1s_v2"],
                                 start=True, stop=False)
                nc.tensor.matmul(vec2, w_wc_hi, sk["xv_sp2"][64:128, :],
                                 start=False, stop=True,
                                 tile_position=(64, 0))
                sk.update(vec01=vec01, vec2=vec2)

            def main_act(k):
                sk = S[k]
                sc_silu = wk.tile([128, NT], F16)
                nc.scalar.activation(sc_silu, sk["scal"], AF.Silu)
                tg = wk.tile([128, NT], F16)
                nc.scalar.activation(tg, sk["gate2"], AF.Tanh, scale=0.5)
                sk.update(sc_silu=sc_silu, tg=tg)

            def out_phase(k):
                """Sigmoid chain + output muls + stores for tile k."""
                sk = S[k]
                sl = slice(k * NT, (k + 1) * NT)
                # sgw2 = (tg+1)*wbs = sigmoid(g)*(w+b)
                sgw2 = wk.tile([128, NT], F16)
                nc.vector.scalar_tensor_tensor(sgw2, sk["tg"], 1.0,
                                               sk["wbs"], ADD, MULT)
                out01 = wk.tile([128, NT], F16)
                nc.vector.tensor_tensor(out01, sk["vec01"], sgw2, MULT)
                out2 = wk.tile([64, NT], F16)
                nc.vector.tensor_tensor(out2, sk["vec2"], sgw2[0:64, :], MULT)
                out_s = wk.tile([128, NT], F16)
                nc.vector.tensor_tensor(out_s, sk["sc_silu"], sk["was"], MULT)
                nc.sync.dma_start(d_out.ap()[128:256, sl], out01)
                nc.gpsimd.dma_start(d_out.ap()[256:320, sl], out2)
                nc.sync.dma_start(d_out.ap()[0:128, sl], out_s)

            # ---- pipelined emission (depth 4) ----------------------
            # iteration k: load(k+1) | prep(k) | h2/TP(k-1) | mlp2+out(k-2)
            T = T_TILES
            for k in range(T + 3):
                if k == 0:
                    load(0)
                    load(1)
                if k + 2 < T:
                    load(k + 2)
                m = k - 1        # TP tile
                o = k - 2        # mlp2 + output tile
                if 0 <= o < T:
                    main_mlp2(o)      # PE 1-2; ACT wbs, was
                    out_phase(o)      # DVE sgw2, out01, out2, out_s
                if k < T:
                    prep_dve(k)       # DVE fused prescales
                    prep(k)           # PE 3: h1; ACT h1s
                if 0 <= m < T:
                    prep_h2(m)        # PE 4: h2; ACT h2s
                    main_tp_a(m)      # PE 5-10
                    main_vec(m)       # PE 11-15
                    main_act(m)       # ACT sc_silu, tg
                if o - 1 in S:
                    del S[o - 1]

    nc.compile()
    _CACHE["nc"] = nc
    return nc


def _fold_weights(inp):
    """Fold per-channel weights + constants into fp16 matmul matrices."""
    f = lambda k: np.asarray(inp[k], dtype=np.float32)
    w0f = f("w1_p0") * f("w2_p0")[None, :] * (INV_S * SQ2)
    w1f = f("w1_p1") * f("w2_p1")[None, :] * (INV_S * SQ2)
    w2f = f("w1_p2") * f("w2_p2")[None, :] * (INV_S * SQ2)
    w3f = f("w1_p3") * f("w2_p3")[None, :] * (INV_V * SQ2)
    w4f = f("w1_p4") * f("w2_p4")[None, :] * (INV_V * SQ3 * SQ2)
    w5f = f("w1_p5") * f("w2_p5")[None, :] * (INV_V * SQ3 * SQ2)
    fc2 = f("fc_w2")
    b2 = f("fc_b2")
    w5d = np.concatenate([w5f, w5f], axis=1)         # [64,128] col-dup
    cdiag = np.zeros((128, 128), np.float32)
    cdiag[0:64, 0:64] = w3f
    cdiag[64:128, 64:128] = w3f
    z64 = np.zeros((128, 64), np.float32)
    h = lambda a: np.ascontiguousarray(a.astype(np.float16))
    c = lambda a: np.ascontiguousarray(a.astype(np.float32))
    return {
        "wa0": h(w0f),
        "wa1d": h(np.concatenate([w1f, w1f], axis=1)),
        "w2lo": h(np.concatenate([w2f, z64], axis=1)),
        "w2hi": h(np.concatenate([z64, w2f], axis=1)),
        "w2v2": h(w2f),
        "wb4s": h(np.concatenate([w4f, w4f], axis=0)),
        "wb5sd": h(np.concatenate([w5d, w5d], axis=0)),
        "wb4b": h(w4f),
        "wb5bd": h(w5d),
        "wcdiag": h(cdiag),
        "wc2": h(w3f),
        "fc0": h(f("fc_w0")),
        "fc1": h(f("fc_w1")),
        "fc2a": h(fc2[:, :128]),
        "fc2bd": h(0.5 * np.concatenate([fc2[:, 128:], fc2[:, 128:]],
                                        axis=1)),
        "b0c": c(f("fc_b0")[:, None]),
        "b1c": c(f("fc_b1")[:, None]),
        "b2a": c(b2[:128, None]),
        "b2bh": c(0.5 * np.concatenate([b2[128:], b2[128:]])[:, None]),
    }


def _shard_inputs(inp):
    """Per-core merged fp16 input tensor + tiny x2 scalar rows."""
    fea_in1 = np.asarray(inp["fea_in1"], dtype=np.float32)
    fea_in2 = np.asarray(inp["fea_in2"], dtype=np.float32)
    fea_w = np.asarray(inp["fea_weight"], dtype=np.float32)
    shards = []
    for cidx in range(N_CORES):
        s = slice(cidx * E_CORE, (cidx + 1) * E_CORE)
        x1 = fea_in1[s]
        x2 = fea_in2[s]
        fw = fea_w[s]
        blocks = np.zeros((N_BLK, 128, E_PAD), np.float16)
        blocks[0][:, :E_CORE] = x1[:, :128].T.astype(np.float16)
        xv = x1[:, 128:].reshape(E_CORE, 64, 3).transpose(2, 1, 0)  # [3,64,E]
        blocks[1][0:64, :E_CORE] = xv[0].astype(np.float16)
        blocks[1][64:128, :E_CORE] = xv[1].astype(np.float16)
        blocks[2][0:64, :E_CORE] = xv[2].astype(np.float16)
        blocks[2][64:128, :E_CORE] = xv[2].astype(np.float16)
        blocks[3][:, :E_CORE] = fw.T.astype(np.float16)
        # x2 rows: [s, v0, v1, v2]
        x2p = np.zeros((4, E_PAD), np.float16)
        x2p[0, :E_CORE] = x2[:, 0].astype(np.float16)
        x2p[1, :E_CORE] = x2[:, 1].astype(np.float16)
        x2p[2, :E_CORE] = x2[:, 2].astype(np.float16)
        x2p[3, :E_CORE] = x2[:, 3].astype(np.float16)
        blocks[4][0:64, :] = x2p[1][None, :]      # r_v01 = [v0; v1]
        blocks[4][64:128, :] = x2p[2][None, :]
        blocks[5][:, :] = x2p[0][None, :]         # r_s full
        blocks[6][:, :] = x2p[1][None, :]         # r_v0 full
        blocks[7][:, :] = x2p[2][None, :]         # r_v1 full
        blocks[8][:, :] = x2p[3][None, :]         # r_v2 full
        blocks[9][0:64, :] = x2p[3][None, :]      # r_sv2 = [v2; s]
        blocks[9][64:128, :] = x2p[0][None, :]
        mega = (blocks.reshape(N_BLK, 128, T_TILES, NT)
                .transpose(1, 2, 0, 3)
                .reshape(128, T_TILES * TILE_COLS))
        shards.append({"in_t": np.ascontiguousarray(mega)})
    return shards


def run(inputs, trace=False, trace_kwargs=None):
    """Run the kernel; returns (output [E,320] f32, BassKernelResults)."""
    _ensure_repo_on_path()
    from concourse import bass_utils

    nc = _build_nc()
    weights = _fold_weights(inputs)
    shards = _shard_inputs(inputs)
    in_maps = [{**weights, **sh} for sh in shards]

    kwargs = {}
    if trace:
        _install_ntff_hook()
        kwargs.update(trace=True, **(trace_kwargs or {}))
    res = bass_utils.run_bass_kernel_spmd(
        nc, in_maps, core_ids=list(range(N_CORES)), **kwargs)

    out = np.empty((E_FULL, 320), np.float32)
    for c in range(N_CORES):
        o = res.results[c]["out_t"][:, :E_CORE].astype(np.float32)
        s = slice(c * E_CORE, (c + 1) * E_CORE)
        out[s, :128] = o[:128].T
        out[s, 128:] = (o[128:].reshape(3, 64, E_CORE)
                        .transpose(2, 1, 0).reshape(E_CORE, 192))
    return out, res


def _install_ntff_hook():
    """Shim the missing antenv.axon_hooks so trace=True works under axon."""
    import types
    import antenv
    from concourse import bass_utils
    if "antenv.axon_hooks" in sys.modules:
        return
    mod = types.ModuleType("antenv.axon_hooks")
    _h = [None]
    mod.set_axon_ntff_profile_hook = lambda h: _h.__setitem__(0, h)
    mod.get_axon_ntff_profile_hook = lambda: _h[0]
    sys.modules["antenv.axon_hooks"] = mod
    antenv.axon_hooks = mod
    from trn_agent_boot.trn_boot import _ntff_profile_via_ctypes
    mod.set_axon_ntff_profile_hook(
        _ntff_profile_via_ctypes("/opt/axon/libaxon_pjrt.so"))
    bass_utils.upload_artifacts = lambda tmpdir: tmpdir


def kernel(**inputs) -> np.ndarray:
    out, _ = run(inputs, trace=False)
    return out


# revision 25
# speedup vs baseline: 1.5793x; 1.0016x over previous
"""EquiConv (DeepH-E3) Trainium2 kernel — 8-core data-parallel over edges.

Strategy (channel-major, fp16 on-device, 4-deep software pipeline):
  - Host folds per-channel weights/constants into fp16 matmul matrices,
    shards edges across 8 cores, pads to 49x512 per core, and packs all
    ten per-tile input row-blocks (x1 scalars, x1 vectors, fea_weight,
    and the 128-replicated per-edge x2 scalars) into ONE interleaved
    DRAM tensor so each tile needs a single load DMA (prefetched two
    iterations ahead).
  - The P2 path uses zero-padded weight columns ([w2|0], [0|w2]) so both
    vector components accumulate into one diag(w3,w3)-anchored PSUM
    group; gate and MLP-wb outputs are column-duplicated so the sigmoid
    gate covers all 128 partitions in single ops.
  - Emission is software-pipelined four tiles deep so every PE matmul's
    inputs are ready >=1 iteration ahead: the PE stays dense and ramps
    to its max p-state (~216-330ns per 512-col fp16 matmul vs 427ns).
  - All SBUF traffic is fp16 (2x DVE rate, operands packed pairwise
    into 1024-col fused DVE ops via stride-0 repeat APs); PSUM stays
    f32.  ACT does the PSUM->SBUF crossings; sigmoid uses tanh(g/2).
  - Output is written fp16 channel-major [320, E] and unpacked on host.

Self-contained: hardcodes shapes from the problem spec; no file reads.
"""
import os
import sys

import numpy as np

# ---------------------------------------------------------------- constants
E_FULL = 200000
N_CORES = 8
E_CORE = E_FULL // N_CORES      # 25000
NT = 512                        # edges per tile
T_TILES = 49                    # tiles per core
E_PAD = NT * T_TILES            # 25088
MUL_S = 128
MUL_V = 64
N_BLK = 10                      # input row-blocks per tile
TILE_COLS = N_BLK * NT          # 5120

INV_S = 1.0 / np.sqrt(MUL_S)
INV_V = 1.0 / np.sqrt(MUL_V)
SQ2 = 1.0 / np.sqrt(2.0)
SQ3 = 1.0 / np.sqrt(3.0)

_REPO_CANDIDATES = (
    "/opt/trn_rl_repo",
    "/root/.axon_site/_ro/trn_rl_repo",
)


def _ensure_repo_on_path():
    try:
        import concourse.bass  # noqa: F401
        return
    except ImportError:
        pass
    for p in _REPO_CANDIDATES:
        if os.path.isdir(p) and p not in sys.path:
            sys.path.insert(0, p)
    import concourse.bass  # noqa: F401


_CACHE = {}


def _build_nc():
    """Build + compile the per-core Bass program (cached)."""
    if "nc" in _CACHE:
        return _CACHE["nc"]
    _ensure_repo_on_path()
    import concourse.mybir as mybir
    import concourse.tile as tile
    from concourse import bacc

    F32 = mybir.dt.float32
    F16 = mybir.dt.float16
    MULT = mybir.AluOpType.mult
    ADD = mybir.AluOpType.add
    AF = mybir.ActivationFunctionType

    nc = bacc.Bacc(trn_type="TRN2", target_bir_lowering=False, debug=False,
                   num_devices=N_CORES)

    # DRAM inputs (per-core shard) ----------------------------------------
    d_in = nc.dram_tensor("in_t", [128, T_TILES * TILE_COLS], F16,
                          kind="ExternalInput")
    d_wa0 = nc.dram_tensor("wa0", [128, 128], F16, kind="ExternalInput")
    d_wa1d = nc.dram_tensor("wa1d", [128, 128], F16, kind="ExternalInput")
    d_w2lo = nc.dram_tensor("w2lo", [128, 128], F16, kind="ExternalInput")
    d_w2hi = nc.dram_tensor("w2hi", [128, 128], F16, kind="ExternalInput")
    d_w2v2 = nc.dram_tensor("w2v2", [128, 64], F16, kind="ExternalInput")
    d_wb4s = nc.dram_tensor("wb4s", [128, 128], F16, kind="ExternalInput")
    d_wb5sd = nc.dram_tensor("wb5sd", [128, 128], F16, kind="ExternalInput")
    d_wb4b = nc.dram_tensor("wb4b", [64, 128], F16, kind="ExternalInput")
    d_wb5bd = nc.dram_tensor("wb5bd", [64, 128], F16, kind="ExternalInput")
    d_wcdiag = nc.dram_tensor("wcdiag", [128, 128], F16, kind="ExternalInput")
    d_wc2 = nc.dram_tensor("wc2", [64, 64], F16, kind="ExternalInput")
    d_fc0 = nc.dram_tensor("fc0", [128, 64], F16, kind="ExternalInput")
    d_fc1 = nc.dram_tensor("fc1", [64, 64], F16, kind="ExternalInput")
    d_fc2a = nc.dram_tensor("fc2a", [64, 128], F16, kind="ExternalInput")
    d_fc2bd = nc.dram_tensor("fc2bd", [64, 128], F16, kind="ExternalInput")
    d_b0 = nc.dram_tensor("b0c", [64, 1], F32, kind="ExternalInput")
    d_b1 = nc.dram_tensor("b1c", [64, 1], F32, kind="ExternalInput")
    d_b2a = nc.dram_tensor("b2a", [128, 1], F32, kind="ExternalInput")
    d_b2bh = nc.dram_tensor("b2bh", [128, 1], F32, kind="ExternalInput")

    d_out = nc.dram_tensor("out_t", [320, E_PAD], F16, kind="ExternalOutput")

    with tile.TileContext(nc) as tc:
        with tc.tile_pool(name="const", bufs=1) as cp, \
             tc.tile_pool(name="io", bufs=4) as io, \
             tc.tile_pool(name="work", bufs=2) as wk, \
             tc.tile_pool(name="ps", bufs=1, space="PSUM") as ps:

            def const(d, shape, dtype=F16):
                t = cp.tile(shape, dtype, name=d.name + "_sb")
                nc.sync.dma_start(t, d.ap())
                return t

            w_wa0 = const(d_wa0, [128, 128])
            w_wa1d = const(d_wa1d, [128, 128])
            w_w2lo = const(d_w2lo, [128, 128])
            w_w2hi = const(d_w2hi, [128, 128])
            w_w2v2 = const(d_w2v2, [128, 64])
            w_wb4s = const(d_wb4s, [128, 128])
            w_wb5sd = const(d_wb5sd, [128, 128])
            w_wb4b = const(d_wb4b, [64, 128])
            w_wb5bd = const(d_wb5bd, [64, 128])
            w_wcdiag = const(d_wcdiag, [128, 128])
            w_fc0 = const(d_fc0, [128, 64])
            w_fc1 = const(d_fc1, [64, 64])
            w_fc2a = const(d_fc2a, [64, 128])
            w_fc2bd = const(d_fc2bd, [64, 128])
            c_b0 = const(d_b0, [64, 1], F32)
            c_b1 = const(d_b1, [64, 1], F32)
            c_b2a = const(d_b2a, [128, 1], F32)
            c_b2bh = const(d_b2bh, [128, 1], F32)
            # w3f copy at partitions 64-127 for the row-offset matmul
            w_wc_f = cp.tile([128, 64], F16, name="wc_hi_sb")
            w_wc_hi = w_wc_f[64:128, :]
            nc.sync.dma_start(w_wc_hi, d_wc2.ap())

            # per-tile state rings, keyed by tile index
            S = {}

            def load(k):
                csl = slice(k * TILE_COLS, (k + 1) * TILE_COLS)
                mega = io.tile([128, TILE_COLS], F16)
                nc.sync.dma_start(mega, d_in.ap()[:, csl])
                S[k] = {"mega": mega}

            def mega_slices(k):
                m = S[k]["mega"]
                return {
                    "x1s": m[:, 0 * NT:1 * NT],
                    "x1va": m[:, 1 * NT:2 * NT],
                    "x2d": m[:, 2 * NT:3 * NT],
                    "fwt": m[:, 3 * NT:4 * NT],
                    "rp01": m[:, 4 * NT:6 * NT],   # [r_v01 | r_s]
                    "rsv0": m[:, 5 * NT:7 * NT],   # [r_s | r_v0]
                    "rv12": m[:, 7 * NT:9 * NT],   # [r_v1 | r_v2]
                    "r_sv2": m[:, 9 * NT:10 * NT],
                }

            def rep2(ap):
                """Stride-0 repeat of a [128, NT] slice -> [128, 2, NT]."""
                return ap.unsqueeze(1).broadcast_to([128, 2, NT])

            def prep_dve(k):
                sk = S[k]
                ms = mega_slices(k)
                f1 = wk.tile([128, 2 * NT], F16)
                nc.vector.tensor_tensor(
                    f1.rearrange("p (b c) -> p b c", b=2),
                    rep2(ms["x1va"]), ms["rp01"].rearrange(
                        "p (b c) -> p b c", b=2), MULT)
                sk.update(xv_p01=f1[:, 0:NT], xv_s01=f1[:, NT:2 * NT])
                f2 = wk.tile([128, 2 * NT], F16)
                nc.vector.tensor_tensor(
                    f2.rearrange("p (b c) -> p b c", b=2),
                    rep2(ms["x1s"]), ms["rsv0"].rearrange(
                        "p (b c) -> p b c", b=2), MULT)
                f3 = wk.tile([128, 2 * NT], F16)
                nc.vector.tensor_tensor(
                    f3.rearrange("p (b c) -> p b c", b=2),
                    rep2(ms["x1s"]), ms["rv12"].rearrange(
                        "p (b c) -> p b c", b=2), MULT)
                xv_sp2 = wk.tile([128, NT], F16)
                nc.vector.tensor_tensor(xv_sp2, ms["x2d"], ms["r_sv2"], MULT)
                sk.update(x1s_s=f2[:, 0:NT], x1s_v0=f2[:, NT:2 * NT],
                          x1s_v1=f3[:, 0:NT], x1s_v2=f3[:, NT:2 * NT],
                          xv_sp2=xv_sp2)

            def prep(k):
                """MLP front (h1) for tile k."""
                sk = S[k]
                ms = mega_slices(k)
                h1 = ps.tile([64, NT], F32, tag="h1")
                nc.tensor.matmul(h1, w_fc0, ms["fwt"], start=True, stop=True)
                h1s = wk.tile([64, NT], F16)
                nc.scalar.activation(h1s, h1, AF.Silu, bias=c_b0)
                sk.update(h1s=h1s)

            def prep_h2(k):
                sk = S[k]
                h2 = ps.tile([64, NT], F32, tag="h2")
                nc.tensor.matmul(h2, w_fc1, sk["h1s"], start=True, stop=True)
                h2s = wk.tile([64, NT], F16)
                nc.scalar.activation(h2s, h2, AF.Silu, bias=c_b1)
                sk.update(h2s=h2s)

            def main_mlp2(k):
                """wwa / wwb matmuls + crossings for tile k."""
                sk = S[k]
                wwa = ps.tile([128, NT], F32, tag="wwa")
                nc.tensor.matmul(wwa, w_fc2a, sk["h2s"], start=True, stop=True)
                wwb = ps.tile([128, NT], F32, tag="wwb")
                nc.tensor.matmul(wwb, w_fc2bd, sk["h2s"],
                                 start=True, stop=True)
                wbs = wk.tile([128, NT], F16)
                nc.scalar.activation(wbs, wwb, AF.Identity, bias=c_b2bh)
                was = wk.tile([128, NT], F16)
                nc.scalar.activation(was, wwa, AF.Identity, bias=c_b2a)
                sk.update(wbs=wbs, was=was)

            def main_tp_a(k):
                """Six accumulating TP matmuls for tile k."""
                sk = S[k]
                scal = ps.tile([128, NT], F32, tag="scal")
                gate2 = ps.tile([128, NT], F32, tag="gate2")
                nc.tensor.matmul(scal, w_wa0, sk["x1s_s"],
                                 start=True, stop=False)
                nc.tensor.matmul(gate2, w_wa1d, sk["x1s_s"],
                                 start=True, stop=False)
                nc.tensor.matmul(scal, w_wb4s, sk["xv_p01"],
                                 start=False, stop=False)
                nc.tensor.matmul(gate2, w_wb5sd, sk["xv_p01"],
                                 start=False, stop=False)
                nc.tensor.matmul(scal, w_wb4b, sk["xv_sp2"][0:64, :],
                                 start=False, stop=True)
                nc.tensor.matmul(gate2, w_wb5bd, sk["xv_sp2"][0:64, :],
                                 start=False, stop=True)
                sk.update(scal=scal, gate2=gate2)

            def main_vec(k):
                """vec01 / vec2 accumulation groups."""
                sk = S[k]
                vec01 = ps.tile([128, NT], F32, tag="vec01")
                vec2 = ps.tile([64, NT], F32, tag="vec2")
                nc.tensor.matmul(vec01, w_wcdiag, sk["xv_s01"],
                                 start=True, stop=False)
                nc.tensor.matmul(vec01, w_w2lo, sk["x1s_v0"],
                                 start=False, stop=False)
                nc.tensor.matmul(vec01, w_w2hi, sk["x1s_v1"],
                                 start=False, stop=True)
                nc.tensor.matmul(vec2, w_w2v2, sk["x1s_v2"],
                                 start=True, stop=False)
                nc.tensor.matmul(vec2, w_wc_hi, sk["xv_sp2"][64:128, :],
                                 start=False, stop=True,
                                 tile_position=(64, 0))
                sk.update(vec01=vec01, vec2=vec2)

            def main_act(k):
                sk = S[k]
                sc_silu = wk.tile([128, NT], F16)
                nc.scalar.activation(sc_silu, sk["scal"], AF.Silu)
                tg = wk.tile([128, NT], F16)
                nc.scalar.activation(tg, sk["gate2"], AF.Tanh, scale=0.5)
                sk.update(sc_silu=sc_silu, tg=tg)

            def out_phase(k):
                """Sigmoid chain + output muls + stores for tile k."""
                sk = S[k]
                sl = slice(k * NT, (k + 1) * NT)
                # sgw2 = (tg+1)*wbs = sigmoid(g)*(w+b)
                sgw2 = wk.tile([128, NT], F16)
                nc.vector.scalar_tensor_tensor(sgw2, sk["tg"], 1.0,
                                               sk["wbs"], ADD, MULT)
                out01 = wk.tile([128, NT], F16)
                nc.vector.tensor_tensor(out01, sk["vec01"], sgw2, MULT)
                out2 = wk.tile([64, NT], F16)
                nc.vector.tensor_tensor(out2, sk["vec2"], sgw2[0:64, :], MULT)
                out_s = wk.tile([128, NT], F16)
                nc.vector.tensor_tensor(out_s, sk["sc_silu"], sk["was"], MULT)
                nc.sync.dma_start(d_out.ap()[128:256, sl], out01)
                nc.gpsimd.dma_start(d_out.ap()[256:320, sl], out2)
                nc.sync.dma_start(d_out.ap()[0:128, sl], out_s)

            # ---- pipelined emission (depth 4) ----------------------
            # iteration k: load(k+1) | prep(k) | h2/TP(k-1) | mlp2+out(k-2)
            T = T_TILES
            for k in range(T + 3):
                if k == 0:
                    load(0)
                    load(1)
                if k + 2 < T:
                    load(k + 2)
                m = k - 1        # TP tile
                o = k - 2        # output tile
                if 0 <= o < T:
                    out_phase(o)      # DVE sgw2, out01, out2, out_s
                if k < T:
                    prep_dve(k)       # DVE fused prescales
                    prep(k)           # PE 1: h1; ACT h1s
                if 0 <= m < T:
                    prep_h2(m)        # PE 2: h2; ACT h2s
                    main_tp_a(m)      # PE 3-8
                    main_vec(m)       # PE 9-13
                    main_act(m)       # ACT sc_silu, tg
                    main_mlp2(m)      # PE 14-15; ACT wbs, was (for k+1)
                if o - 1 in S:
                    del S[o - 1]

    nc.compile()
    _CACHE["nc"] = nc
    return nc


def _fold_weights(inp):
    """Fold per-channel weights + constants into fp16 matmul matrices."""
    f = lambda k: np.asarray(inp[k], dtype=np.float32)
    w0f = f("w1_p0") * f("w2_p0")[None, :] * (INV_S * SQ2)
    w1f = f("w1_p1") * f("w2_p1")[None, :] * (INV_S * SQ2)
    w2f = f("w1_p2") * f("w2_p2")[None, :] * (INV_S * SQ2)
    w3f = f("w1_p3") * f("w2_p3")[None, :] * (INV_V * SQ2)
    w4f = f("w1_p4") * f("w2_p4")[None, :] * (INV_V * SQ3 * SQ2)
    w5f = f("w1_p5") * f("w2_p5")[None, :] * (INV_V * SQ3 * SQ2)
    fc2 = f("fc_w2")
    b2 = f("fc_b2")
    w5d = np.concatenate([w5f, w5f], axis=1)         # [64,128] col-dup
    cdiag = np.zeros((128, 128), np.float32)
    cdiag[0:64, 0:64] = w3f
    cdiag[64:128, 64:128] = w3f
    z64 = np.zeros((128, 64), np.float32)
    h = lambda a: np.ascontiguousarray(a.astype(np.float16))
    c = lambda a: np.ascontiguousarray(a.astype(np.float32))
    return {
        "wa0": h(w0f),
        "wa1d": h(np.concatenate([w1f, w1f], axis=1)),
        "w2lo": h(np.concatenate([w2f, z64], axis=1)),
        "w2hi": h(np.concatenate([z64, w2f], axis=1)),
        "w2v2": h(w2f),
        "wb4s": h(np.concatenate([w4f, w4f], axis=0)),
        "wb5sd": h(np.concatenate([w5d, w5d], axis=0)),
        "wb4b": h(w4f),
        "wb5bd": h(w5d),
        "wcdiag": h(cdiag),
        "wc2": h(w3f),
        "fc0": h(f("fc_w0")),
        "fc1": h(f("fc_w1")),
        "fc2a": h(fc2[:, :128]),
        "fc2bd": h(0.5 * np.concatenate([fc2[:, 128:], fc2[:, 128:]],
                                        axis=1)),
        "b0c": c(f("fc_b0")[:, None]),
        "b1c": c(f("fc_b1")[:, None]),
        "b2a": c(b2[:128, None]),
        "b2bh": c(0.5 * np.concatenate([b2[128:], b2[128:]])[:, None]),
    }


def _shard_inputs(inp):
    """Per-core merged fp16 input tensor + tiny x2 scalar rows."""
    fea_in1 = np.asarray(inp["fea_in1"], dtype=np.float32)
    fea_in2 = np.asarray(inp["fea_in2"], dtype=np.float32)
    fea_w = np.asarray(inp["fea_weight"], dtype=np.float32)
    shards = []
    for cidx in range(N_CORES):
        s = slice(cidx * E_CORE, (cidx + 1) * E_CORE)
        x1 = fea_in1[s]
        x2 = fea_in2[s]
        fw = fea_w[s]
        blocks = np.zeros((N_BLK, 128, E_PAD), np.float16)
        blocks[0][:, :E_CORE] = x1[:, :128].T.astype(np.float16)
        xv = x1[:, 128:].reshape(E_CORE, 64, 3).transpose(2, 1, 0)  # [3,64,E]
        blocks[1][0:64, :E_CORE] = xv[0].astype(np.float16)
        blocks[1][64:128, :E_CORE] = xv[1].astype(np.float16)
        blocks[2][0:64, :E_CORE] = xv[2].astype(np.float16)
        blocks[2][64:128, :E_CORE] = xv[2].astype(np.float16)
        blocks[3][:, :E_CORE] = fw.T.astype(np.float16)
        # x2 rows: [s, v0, v1, v2]
        x2p = np.zeros((4, E_PAD), np.float16)
        x2p[0, :E_CORE] = x2[:, 0].astype(np.float16)
        x2p[1, :E_CORE] = x2[:, 1].astype(np.float16)
        x2p[2, :E_CORE] = x2[:, 2].astype(np.float16)
        x2p[3, :E_CORE] = x2[:, 3].astype(np.float16)
        blocks[4][0:64, :] = x2p[1][None, :]      # r_v01 = [v0; v1]
        blocks[4][64:128, :] = x2p[2][None, :]
        blocks[5][:, :] = x2p[0][None, :]         # r_s full
        blocks[6][:, :] = x2p[1][None, :]         # r_v0 full
        blocks[7][:, :] = x2p[2][None, :]         # r_v1 full
        blocks[8][:, :] = x2p[3][None, :]         # r_v2 full
        blocks[9][0:64, :] = x2p[3][None, :]      # r_sv2 = [v2; s]
        blocks[9][64:128, :] = x2p[0][None, :]
        mega = (blocks.reshape(N_BLK, 128, T_TILES, NT)
                .transpose(1, 2, 0, 3)
                .reshape(128, T_TILES * TILE_COLS))
        shards.append({"in_t": np.ascontiguousarray(mega)})
    return shards


def run(inputs, trace=False, trace_kwargs=None):
    """Run the kernel; returns (output [E,320] f32, BassKernelResults)."""
    _ensure_repo_on_path()
    from concourse import bass_utils

    nc = _build_nc()
    weights = _fold_weights(inputs)
    shards = _shard_inputs(inputs)
    in_maps = [{**weights, **sh} for sh in shards]

    kwargs = {}
    if trace:
        _install_ntff_hook()
        kwargs.update(trace=True, **(trace_kwargs or {}))
    res = bass_utils.run_bass_kernel_spmd(
        nc, in_maps, core_ids=list(range(N_CORES)), **kwargs)

    out = np.empty((E_FULL, 320), np.float32)
    for c in range(N_CORES):
        o = res.results[c]["out_t"][:, :E_CORE].astype(np.float32)
        s = slice(c * E_CORE, (c + 1) * E_CORE)
        out[s, :128] = o[:128].T
        out[s, 128:] = (o[128:].reshape(3, 64, E_CORE)
                        .transpose(2, 1, 0).reshape(E_CORE, 192))
    return out, res


def _install_ntff_hook():
    """Shim the missing antenv.axon_hooks so trace=True works under axon."""
    import types
    import antenv
    from concourse import bass_utils
    if "antenv.axon_hooks" in sys.modules:
        return
    mod = types.ModuleType("antenv.axon_hooks")
    _h = [None]
    mod.set_axon_ntff_profile_hook = lambda h: _h.__setitem__(0, h)
    mod.get_axon_ntff_profile_hook = lambda: _h[0]
    sys.modules["antenv.axon_hooks"] = mod
    antenv.axon_hooks = mod
    from trn_agent_boot.trn_boot import _ntff_profile_via_ctypes
    mod.set_axon_ntff_profile_hook(
        _ntff_profile_via_ctypes("/opt/axon/libaxon_pjrt.so"))
    bass_utils.upload_artifacts = lambda tmpdir: tmpdir


def kernel(**inputs) -> np.ndarray:
    out, _ = run(inputs, trace=False)
    return out


# revision 26
# speedup vs baseline: 1.6053x; 1.0164x over previous
"""EquiConv (DeepH-E3) Trainium2 kernel — 8-core data-parallel over edges.

Strategy (channel-major, fp16 on-device, 4-deep software pipeline):
  - Host folds per-channel weights/constants into fp16 matmul matrices,
    shards edges across 8 cores, pads to 49x512 per core, and packs all
    ten per-tile input row-blocks (x1 scalars, x1 vectors, fea_weight,
    and the 128-replicated per-edge x2 scalars) into ONE interleaved
    DRAM tensor so each tile needs a single load DMA (prefetched two
    iterations ahead).
  - The P2 path uses zero-padded weight columns ([w2|0], [0|w2]) so both
    vector components accumulate into one diag(w3,w3)-anchored PSUM
    group; gate and MLP-wb outputs are column-duplicated so the sigmoid
    gate covers all 128 partitions in single ops.
  - Emission is software-pipelined four tiles deep so every PE matmul's
    inputs are ready >=1 iteration ahead: the PE stays dense and ramps
    to its max p-state (~216-330ns per 512-col fp16 matmul vs 427ns).
  - All SBUF traffic is fp16 (2x DVE rate, operands packed pairwise
    into 1024-col fused DVE ops via stride-0 repeat APs); PSUM stays
    f32.  ACT does the PSUM->SBUF crossings; sigmoid uses tanh(g/2).
  - Output is written fp16 channel-major [320, E] and unpacked on host.

Self-contained: hardcodes shapes from the problem spec; no file reads.
"""
import os
import sys

import numpy as np

# ---------------------------------------------------------------- constants
E_FULL = 200000
N_CORES = 8
E_CORE = E_FULL // N_CORES      # 25000
NT = 512                        # edges per tile
T_TILES = 49                    # tiles per core
E_PAD = NT * T_TILES            # 25088
MUL_S = 128
MUL_V = 64
N_BLK = 10                      # input row-blocks per tile
TILE_COLS = N_BLK * NT          # 5120

INV_S = 1.0 / np.sqrt(MUL_S)
INV_V = 1.0 / np.sqrt(MUL_V)
SQ2 = 1.0 / np.sqrt(2.0)
SQ3 = 1.0 / np.sqrt(3.0)

_REPO_CANDIDATES = (
    "/opt/trn_rl_repo",
    "/root/.axon_site/_ro/trn_rl_repo",
)


def _ensure_repo_on_path():
    try:
        import concourse.bass  # noqa: F401
        return
    except ImportError:
        pass
    for p in _REPO_CANDIDATES:
        if os.path.isdir(p) and p not in sys.path:
            sys.path.insert(0, p)
    import concourse.bass  # noqa: F401


_CACHE = {}


def _build_nc():
    """Build + compile the per-core Bass program (cached)."""
    if "nc" in _CACHE:
        return _CACHE["nc"]
    _ensure_repo_on_path()
    import concourse.mybir as mybir
    import concourse.tile as tile
    from concourse import bacc

    F32 = mybir.dt.float32
    F16 = mybir.dt.float16
    MULT = mybir.AluOpType.mult
    ADD = mybir.AluOpType.add
    AF = mybir.ActivationFunctionType

    nc = bacc.Bacc(trn_type="TRN2", target_bir_lowering=False, debug=False,
                   num_devices=N_CORES)

    # DRAM inputs (per-core shard) ----------------------------------------
    d_in = nc.dram_tensor("in_t", [128, T_TILES * TILE_COLS], F16,
                          kind="ExternalInput")
    d_wa0 = nc.dram_tensor("wa0", [128, 128], F16, kind="ExternalInput")
    d_wa1d = nc.dram_tensor("wa1d", [128, 128], F16, kind="ExternalInput")
    d_w2lo = nc.dram_tensor("w2lo", [128, 128], F16, kind="ExternalInput")
    d_w2hi = nc.dram_tensor("w2hi", [128, 128], F16, kind="ExternalInput")
    d_w2v2 = nc.dram_tensor("w2v2", [128, 64], F16, kind="ExternalInput")
    d_wb4s = nc.dram_tensor("wb4s", [128, 128], F16, kind="ExternalInput")
    d_wb5sd = nc.dram_tensor("wb5sd", [128, 128], F16, kind="ExternalInput")
    d_wb4b = nc.dram_tensor("wb4b", [64, 128], F16, kind="ExternalInput")
    d_wb5bd = nc.dram_tensor("wb5bd", [64, 128], F16, kind="ExternalInput")
    d_wcdiag = nc.dram_tensor("wcdiag", [128, 128], F16, kind="ExternalInput")
    d_wc2 = nc.dram_tensor("wc2", [64, 64], F16, kind="ExternalInput")
    d_fc0 = nc.dram_tensor("fc0", [128, 64], F16, kind="ExternalInput")
    d_fc1 = nc.dram_tensor("fc1", [64, 64], F16, kind="ExternalInput")
    d_fc2a = nc.dram_tensor("fc2a", [64, 128], F16, kind="ExternalInput")
    d_fc2bd = nc.dram_tensor("fc2bd", [64, 128], F16, kind="ExternalInput")
    d_b0 = nc.dram_tensor("b0c", [64, 1], F32, kind="ExternalInput")
    d_b1 = nc.dram_tensor("b1c", [64, 1], F32, kind="ExternalInput")
    d_b2a = nc.dram_tensor("b2a", [128, 1], F32, kind="ExternalInput")
    d_b2bh = nc.dram_tensor("b2bh", [128, 1], F32, kind="ExternalInput")

    d_out = nc.dram_tensor("out_t", [320, E_PAD], F16, kind="ExternalOutput")

    with tile.TileContext(nc) as tc:
        with tc.tile_pool(name="const", bufs=1) as cp, \
             tc.tile_pool(name="io", bufs=4) as io, \
             tc.tile_pool(name="work", bufs=2) as wk, \
             tc.tile_pool(name="ps", bufs=1, space="PSUM") as ps:

            def const(d, shape, dtype=F16):
                t = cp.tile(shape, dtype, name=d.name + "_sb")
                nc.sync.dma_start(t, d.ap())
                return t

            w_wa0 = const(d_wa0, [128, 128])
            w_wa1d = const(d_wa1d, [128, 128])
            w_w2lo = const(d_w2lo, [128, 128])
            w_w2hi = const(d_w2hi, [128, 128])
            w_w2v2 = const(d_w2v2, [128, 64])
            w_wb4s = const(d_wb4s, [128, 128])
            w_wb5sd = const(d_wb5sd, [128, 128])
            w_wb4b = const(d_wb4b, [64, 128])
            w_wb5bd = const(d_wb5bd, [64, 128])
            w_wcdiag = const(d_wcdiag, [128, 128])
            w_fc0 = const(d_fc0, [128, 64])
            w_fc1 = const(d_fc1, [64, 64])
            w_fc2a = const(d_fc2a, [64, 128])
            w_fc2bd = const(d_fc2bd, [64, 128])
            c_b0 = const(d_b0, [64, 1], F32)
            c_b1 = const(d_b1, [64, 1], F32)
            c_b2a = const(d_b2a, [128, 1], F32)
            c_b2bh = const(d_b2bh, [128, 1], F32)
            # w3f copy at partitions 64-127 for the row-offset matmul
            w_wc_f = cp.tile([128, 64], F16, name="wc_hi_sb")
            w_wc_hi = w_wc_f[64:128, :]
            nc.sync.dma_start(w_wc_hi, d_wc2.ap())

            # per-tile state rings, keyed by tile index
            S = {}

            def load(k):
                csl = slice(k * TILE_COLS, (k + 1) * TILE_COLS)
                mega = io.tile([128, TILE_COLS], F16)
                nc.sync.dma_start(mega, d_in.ap()[:, csl])
                S[k] = {"mega": mega}

            def mega_slices(k):
                m = S[k]["mega"]
                return {
                    "x1s": m[:, 0 * NT:1 * NT],
                    "x1va": m[:, 1 * NT:2 * NT],
                    "x2d": m[:, 2 * NT:3 * NT],
                    "fwt": m[:, 3 * NT:4 * NT],
                    "rp01": m[:, 4 * NT:6 * NT],   # [r_v01 | r_s]
                    "rsv0": m[:, 5 * NT:7 * NT],   # [r_s | r_v0]
                    "rv12": m[:, 7 * NT:9 * NT],   # [r_v1 | r_v2]
                    "r_sv2": m[:, 9 * NT:10 * NT],
                }

            def rep2(ap):
                """Stride-0 repeat of a [128, NT] slice -> [128, 2, NT]."""
                return ap.unsqueeze(1).broadcast_to([128, 2, NT])

            def prep_dve(k):
                sk = S[k]
                ms = mega_slices(k)
                f1 = wk.tile([128, 2 * NT], F16)
                nc.vector.tensor_tensor(
                    f1.rearrange("p (b c) -> p b c", b=2),
                    rep2(ms["x1va"]), ms["rp01"].rearrange(
                        "p (b c) -> p b c", b=2), MULT)
                sk.update(xv_p01=f1[:, 0:NT], xv_s01=f1[:, NT:2 * NT])
                f2 = wk.tile([128, 2 * NT], F16)
                nc.vector.tensor_tensor(
                    f2.rearrange("p (b c) -> p b c", b=2),
                    rep2(ms["x1s"]), ms["rsv0"].rearrange(
                        "p (b c) -> p b c", b=2), MULT)
                f3 = wk.tile([128, 2 * NT], F16)
                nc.vector.tensor_tensor(
                    f3.rearrange("p (b c) -> p b c", b=2),
                    rep2(ms["x1s"]), ms["rv12"].rearrange(
                        "p (b c) -> p b c", b=2), MULT)
                xv_sp2 = wk.tile([128, NT], F16)
                nc.vector.tensor_tensor(xv_sp2, ms["x2d"], ms["r_sv2"], MULT)
                sk.update(x1s_s=f2[:, 0:NT], x1s_v0=f2[:, NT:2 * NT],
                          x1s_v1=f3[:, 0:NT], x1s_v2=f3[:, NT:2 * NT],
                          xv_sp2=xv_sp2)

            def prep(k):
                """MLP front (h1) for tile k."""
                sk = S[k]
                ms = mega_slices(k)
                h1 = ps.tile([64, NT], F32, tag="h1")
                nc.tensor.matmul(h1, w_fc0, ms["fwt"], start=True, stop=True)
                h1s = wk.tile([64, NT], F16)
                nc.scalar.activation(h1s, h1, AF.Silu, bias=c_b0)
                sk.update(h1s=h1s)

            def prep_h2(k):
                sk = S[k]
                h2 = ps.tile([64, NT], F32, tag="h2")
                nc.tensor.matmul(h2, w_fc1, sk["h1s"], start=True, stop=True)
                h2s = wk.tile([64, NT], F16)
                nc.scalar.activation(h2s, h2, AF.Silu, bias=c_b1)
                sk.update(h2s=h2s)

            def main_mlp2(k):
                """wwa / wwb matmuls + crossings for tile k."""
                sk = S[k]
                wwa = ps.tile([128, NT], F32, tag="wwa")
                nc.tensor.matmul(wwa, w_fc2a, sk["h2s"], start=True, stop=True)
                wwb = ps.tile([128, NT], F32, tag="wwb")
                nc.tensor.matmul(wwb, w_fc2bd, sk["h2s"],
                                 start=True, stop=True)
                wbs = wk.tile([128, NT], F16)
                nc.scalar.activation(wbs, wwb, AF.Identity, bias=c_b2bh)
                was = wk.tile([128, NT], F16)
                nc.scalar.activation(was, wwa, AF.Identity, bias=c_b2a)
                sk.update(wbs=wbs, was=was)

            def main_tp_a(k):
                """Six accumulating TP matmuls for tile k."""
                sk = S[k]
                scal = ps.tile([128, NT], F32, tag="scal")
                gate2 = ps.tile([128, NT], F32, tag="gate2")
                nc.tensor.matmul(scal, w_wa0, sk["x1s_s"],
                                 start=True, stop=False)
                nc.tensor.matmul(gate2, w_wa1d, sk["x1s_s"],
                                 start=True, stop=False)
                nc.tensor.matmul(scal, w_wb4s, sk["xv_p01"],
                                 start=False, stop=False)
                nc.tensor.matmul(gate2, w_wb5sd, sk["xv_p01"],
                                 start=False, stop=False)
                nc.tensor.matmul(scal, w_wb4b, sk["xv_sp2"][0:64, :],
                                 start=False, stop=True)
                nc.tensor.matmul(gate2, w_wb5bd, sk["xv_sp2"][0:64, :],
                                 start=False, stop=True)
                sk.update(scal=scal, gate2=gate2)

            def main_vec(k):
                """vec01 / vec2 accumulation groups."""
                sk = S[k]
                vec01 = ps.tile([128, NT], F32, tag="vec01")
                vec2 = ps.tile([64, NT], F32, tag="vec2")
                nc.tensor.matmul(vec01, w_wcdiag, sk["xv_s01"],
                                 start=True, stop=False)
                nc.tensor.matmul(vec01, w_w2lo, sk["x1s_v0"],
                                 start=False, stop=False)
                nc.tensor.matmul(vec01, w_w2hi, sk["x1s_v1"],
                                 start=False, stop=True)
                nc.tensor.matmul(vec2, w_w2v2, sk["x1s_v2"],
                                 start=True, stop=False)
                nc.tensor.matmul(vec2, w_wc_hi, sk["xv_sp2"][64:128, :],
                                 start=False, stop=True,
                                 tile_position=(64, 0))
                sk.update(vec01=vec01, vec2=vec2)

            def main_act(k):
                sk = S[k]
                sc_silu = wk.tile([128, NT], F16)
                nc.scalar.activation(sc_silu, sk["scal"], AF.Silu)
                tg = wk.tile([128, NT], F16)
                nc.scalar.activation(tg, sk["gate2"], AF.Tanh, scale=0.5)
                sk.update(sc_silu=sc_silu, tg=tg)

            def out_phase(k):
                """Sigmoid chain + output muls + stores for tile k."""
                sk = S[k]
                sl = slice(k * NT, (k + 1) * NT)
                # sgw2 = (tg+1)*wbs = sigmoid(g)*(w+b)
                tga = wk.tile([128, NT], F16)
                nc.vector.tensor_scalar_add(tga, sk["tg"], 1.0)
                sgw2 = wk.tile([128, NT], F16)
                nc.vector.tensor_tensor(sgw2, tga, sk["wbs"], MULT)
                out01 = wk.tile([128, NT], F16)
                nc.vector.tensor_tensor(out01, sk["vec01"], sgw2, MULT)
                out2 = wk.tile([64, NT], F16)
                nc.vector.tensor_tensor(out2, sk["vec2"], sgw2[0:64, :], MULT)
                out_s = wk.tile([128, NT], F16)
                nc.vector.tensor_tensor(out_s, sk["sc_silu"], sk["was"], MULT)
                nc.sync.dma_start(d_out.ap()[128:256, sl], out01)
                nc.gpsimd.dma_start(d_out.ap()[256:320, sl], out2)
                nc.sync.dma_start(d_out.ap()[0:128, sl], out_s)

            # ---- pipelined emission (depth 4) ----------------------
            # iteration k: load(k+1) | prep(k) | h2/TP(k-1) | mlp2+out(k-2)
            T = T_TILES
            for k in range(T + 3):
                if k == 0:
                    load(0)
                    load(1)
                if k + 2 < T:
                    load(k + 2)
                m = k - 1        # TP tile
                o = k - 2        # output tile
                if 0 <= o < T:
                    out_phase(o)      # DVE sgw2, out01, out2, out_s
                if k < T:
                    prep_dve(k)       # DVE fused prescales
                    prep(k)           # PE 1: h1; ACT h1s
                if 0 <= m < T:
                    prep_h2(m)        # PE 2: h2; ACT h2s
                    main_tp_a(m)      # PE 3-8
                    main_vec(m)       # PE 9-13
                    main_act(m)       # ACT sc_silu, tg
                    main_mlp2(m)      # PE 14-15; ACT wbs, was (for k+1)
                if o - 1 in S:
                    del S[o - 1]

    nc.compile()
    _CACHE["nc"] = nc
    return nc


def _fold_weights(inp):
    """Fold per-channel weights + constants into fp16 matmul matrices."""
    f = lambda k: np.asarray(inp[k], dtype=np.float32)
    w0f = f("w1_p0") * f("w2_p0")[None, :] * (INV_S * SQ2)
    w1f = f("w1_p1") * f("w2_p1")[None, :] * (INV_S * SQ2)
    w2f = f("w1_p2") * f("w2_p2")[None, :] * (INV_S * SQ2)
    w3f = f("w1_p3") * f("w2_p3")[None, :] * (INV_V * SQ2)
    w4f = f("w1_p4") * f("w2_p4")[None, :] * (INV_V * SQ3 * SQ2)
    w5f = f("w1_p5") * f("w2_p5")[None, :] * (INV_V * SQ3 * SQ2)
    fc2 = f("fc_w2")
    b2 = f("fc_b2")
    w5d = np.concatenate([w5f, w5f], axis=1)         # [64,128] col-dup
    cdiag = np.zeros((128, 128), np.float32)
    cdiag[0:64, 0:64] = w3f
    cdiag[64:128, 64:128] = w3f
    z64 = np.zeros((128, 64), np.float32)
    h = lambda a: np.ascontiguousarray(a.astype(np.float16))
    c = lambda a: np.ascontiguousarray(a.astype(np.float32))
    return {
        "wa0": h(w0f),
        "wa1d": h(np.concatenate([w1f, w1f], axis=1)),
        "w2lo": h(np.concatenate([w2f, z64], axis=1)),
        "w2hi": h(np.concatenate([z64, w2f], axis=1)),
        "w2v2": h(w2f),
        "wb4s": h(np.concatenate([w4f, w4f], axis=0)),
        "wb5sd": h(np.concatenate([w5d, w5d], axis=0)),
        "wb4b": h(w4f),
        "wb5bd": h(w5d),
        "wcdiag": h(cdiag),
        "wc2": h(w3f),
        "fc0": h(f("fc_w0")),
        "fc1": h(f("fc_w1")),
        "fc2a": h(fc2[:, :128]),
        "fc2bd": h(0.5 * np.concatenate([fc2[:, 128:], fc2[:, 128:]],
                                        axis=1)),
        "b0c": c(f("fc_b0")[:, None]),
        "b1c": c(f("fc_b1")[:, None]),
        "b2a": c(b2[:128, None]),
        "b2bh": c(0.5 * np.concatenate([b2[128:], b2[128:]])[:, None]),
    }


def _shard_inputs(inp):
    """Per-core merged fp16 input tensor + tiny x2 scalar rows."""
    fea_in1 = np.asarray(inp["fea_in1"], dtype=np.float32)
    fea_in2 = np.asarray(inp["fea_in2"], dtype=np.float32)
    fea_w = np.asarray(inp["fea_weight"], dtype=np.float32)
    shards = []
    for cidx in range(N_CORES):
        s = slice(cidx * E_CORE, (cidx + 1) * E_CORE)
        x1 = fea_in1[s]
        x2 = fea_in2[s]
        fw = fea_w[s]
        blocks = np.zeros((N_BLK, 128, E_PAD), np.float16)
        blocks[0][:, :E_CORE] = x1[:, :128].T.astype(np.float16)
        xv = x1[:, 128:].reshape(E_CORE, 64, 3).transpose(2, 1, 0)  # [3,64,E]
        blocks[1][0:64, :E_CORE] = xv[0].astype(np.float16)
        blocks[1][64:128, :E_CORE] = xv[1].astype(np.float16)
        blocks[2][0:64, :E_CORE] = xv[2].astype(np.float16)
        blocks[2][64:128, :E_CORE] = xv[2].astype(np.float16)
        blocks[3][:, :E_CORE] = fw.T.astype(np.float16)
        # x2 rows: [s, v0, v1, v2]
        x2p = np.zeros((4, E_PAD), np.float16)
        x2p[0, :E_CORE] = x2[:, 0].astype(np.float16)
        x2p[1, :E_CORE] = x2[:, 1].astype(np.float16)
        x2p[2, :E_CORE] = x2[:, 2].astype(np.float16)
        x2p[3, :E_CORE] = x2[:, 3].astype(np.float16)
        blocks[4][0:64, :] = x2p[1][None, :]      # r_v01 = [v0; v1]
        blocks[4][64:128, :] = x2p[2][None, :]
        blocks[5][:, :] = x2p[0][None, :]         # r_s full
        blocks[6][:, :] = x2p[1][None, :]         # r_v0 full
        blocks[7][:, :] = x2p[2][None, :]         # r_v1 full
        blocks[8][:, :] = x2p[3][None, :]         # r_v2 full
        blocks[9][0:64, :] = x2p[3][None, :]      # r_sv2 = [v2; s]
        blocks[9][64:128, :] = x2p[0][None, :]
        mega = (blocks.reshape(N_BLK, 128, T_TILES, NT)
                .transpose(1, 2, 0, 3)
                .reshape(128, T_TILES * TILE_COLS))
        shards.append({"in_t": np.ascontiguousarray(mega)})
    return shards


def run(inputs, trace=False, trace_kwargs=None):
    """Run the kernel; returns (output [E,320] f32, BassKernelResults)."""
    _ensure_repo_on_path()
    from concourse import bass_utils

    nc = _build_nc()
    weights = _fold_weights(inputs)
    shards = _shard_inputs(inputs)
    in_maps = [{**weights, **sh} for sh in shards]

    kwargs = {}
    if trace:
        _install_ntff_hook()
        kwargs.update(trace=True, **(trace_kwargs or {}))
    res = bass_utils.run_bass_kernel_spmd(
        nc, in_maps, core_ids=list(range(N_CORES)), **kwargs)

    out = np.empty((E_FULL, 320), np.float32)
    for c in range(N_CORES):
        o = res.results[c]["out_t"][:, :E_CORE].astype(np.float32)
        s = slice(c * E_CORE, (c + 1) * E_CORE)
        out[s, :128] = o[:128].T
        out[s, 128:] = (o[128:].reshape(3, 64, E_CORE)
                        .transpose(2, 1, 0).reshape(E_CORE, 192))
    return out, res


def _install_ntff_hook():
    """Shim the missing antenv.axon_hooks so trace=True works under axon."""
    import types
    import antenv
    from concourse import bass_utils
    if "antenv.axon_hooks" in sys.modules:
        return
    mod = types.ModuleType("antenv.axon_hooks")
    _h = [None]
    mod.set_axon_ntff_profile_hook = lambda h: _h.__setitem__(0, h)
    mod.get_axon_ntff_profile_hook = lambda: _h[0]
    sys.modules["antenv.axon_hooks"] = mod
    antenv.axon_hooks = mod
    from trn_agent_boot.trn_boot import _ntff_profile_via_ctypes
    mod.set_axon_ntff_profile_hook(
        _ntff_profile_via_ctypes("/opt/axon/libaxon_pjrt.so"))
    bass_utils.upload_artifacts = lambda tmpdir: tmpdir


def kernel(**inputs) -> np.ndarray:
    out, _ = run(inputs, trace=False)
    return out
